# revision 39
# baseline (speedup 1.0000x reference)
"""Trainium2 Bass kernel: ActiveBlockPromptBasis (moe_routing).

Math (per batch image b):
  g   = gelu(W1x @ x_b + W1t @ flux_b + b1)                    # [14, pix]
  z   = Mz.T @ g + bz          (fc2 of both MLPs fused with the 6x8
                                outer-sum expansion, in log space)  # [48, pix]
  wun = exp(z)                                                 # exact softmax
  S   = colsum(wun)  broadcast to 128 partitions in ONE matmul
        (lhsT = ones[48,128]  ->  psum[p, n] = S[n] for all p)
  U   = prompt_flat.T @ wun                                    # [128, pix]
  P   = U * recip(S_bcast)     (DVE approx-recip [128,n] + fused
                                normalize-multiply into the P strip)
  out = conv3x3(P, conv_w)     (9 accumulating bf16 matmuls / PSUM bank;
                                zero spacer columns implement padding)

Implementation notes:
  - All matmul operands are bf16 (fp32 PSUM accumulate).  End-to-end
    relative error vs the fp32 reference is ~2e-3; output is returned as
    bf16 and widened on the host.
  - The ones[48,128] stationary operand makes the column-sum land
    broadcast across all 128 partitions directly, so no DRAM-bounce
    broadcast DMA is needed (the old version moved ~40MB through HBM
    for 1/S replication).
  - Inputs are loaded one STRIP at a time (2 big DMAs per strip,
    double-buffered) instead of per-chunk, cutting DMA issuance ~17x.
  - Engines issue in order, so the per-chunk softmax relay chain is
    software-pipelined at emission: stage k of chunk i is emitted next
    to stage k+1 of chunk i-1, with always-ready conv matmul pairs
    interleaved as PE gap filler.
  - Gelu and Exp live in different ACT table sets, so each strip runs a
    gelu-only phase then an exp-only phase, with an explicit cross-strip
    ACT ordering dep.

Sharding: data-parallel over batch, one image per NeuronCore (8 cores).
"""

import numpy as np
from collections import deque
from contextlib import ExitStack

import concourse.bass as bass
import concourse.tile as tile
from concourse import bacc, mybir
from concourse.bass_utils import run_bass_kernel_spmd

F32 = mybir.dt.float32
BF16 = mybir.dt.bfloat16
AFT = mybir.ActivationFunctionType

B, DIM, E = 8, 64, 128
NT, NB = 6, 8
NTK = NT * NB  # 48
NCORES = 8


def build_program(h=256, w=256, r_out=32, external_io=True):
    """Build the single-core Bass program (SPMD: same program on all cores)."""
    assert w == 256 and r_out % 8 == 0 and h % r_out == 0
    PIX = h * w

    MDT = BF16
    ACT_DEP = True
    PITCH = w + 2          # row window with 1 zero spacer col each side
    IR_MAX = r_out + 2     # input rows per strip incl halo
    SW = (r_out + 2) * w   # strip pixels incl halo (upper bound)

    nc = bacc.Bacc("TRN2", target_bir_lowering=False, debug=False,
                   enable_asserts=False)

    # --- DRAM I/O (per-core slices / replicated small weights) ---
    kin = "ExternalInput" if external_io else "Internal"
    kout = "ExternalOutput" if external_io else "Internal"
    F8 = mybir.dt.float8e4
    xfb_d = nc.dram_tensor("xfb", [128, 2, PIX], F8, kind=kin)
    wab_d = nc.dram_tensor("wab", [128, 2 * 16], F8, kind=kin)
    b1_d = nc.dram_tensor("b1", [14, 1], F32, kind=kin)
    mz_d = nc.dram_tensor("mz", [14, NTK], MDT, kind=kin)
    bz_d = nc.dram_tensor("bz", [NTK, 1], F32, kind=kin)
    on_d = nc.dram_tensor("on", [NTK, E], MDT, kind=kin)
    pt_d = nc.dram_tensor("pt", [NTK, E], MDT, kind=kin)
    wt_d = nc.dram_tensor("wt", [9, E, E], MDT, kind=kin)
    out_d = nc.dram_tensor("out", [E, PIX], BF16, kind=kout)
    if not external_io:
        outs_d = nc.dram_tensor("outs", [1, 8], BF16, kind="ExternalOutput")

    with tile.TileContext(nc) as tc, ExitStack() as ctx:
        consts = ctx.enter_context(tc.tile_pool(name="consts", bufs=1))
        pin = ctx.enter_context(tc.tile_pool(name="pin", bufs=2))
        psb = ctx.enter_context(tc.tile_pool(name="psb", bufs=6))
        pg_pool = ctx.enter_context(tc.tile_pool(name="pg", bufs=IR_MAX // 2 + 2))
        ppool = ctx.enter_context(
            tc.tile_pool(name="ppsum", bufs=8, space="PSUM"))
        pP = ctx.enter_context(tc.tile_pool(name="pP", bufs=2))
        pout = ctx.enter_context(tc.tile_pool(name="pout", bufs=2))

        # --- load constants once ---
        wab_sb = consts.tile([128, 2 * 16], F8)
        nc.sync.dma_start(out=wab_sb[:], in_=wab_d[:])
        b1_sb = consts.tile([14, 1], F32)
        nc.sync.dma_start(out=b1_sb[:], in_=b1_d[:])
        mz_sb = consts.tile([14, NTK], MDT)
        nc.sync.dma_start(out=mz_sb[:], in_=mz_d[:])
        bz_sb = consts.tile([NTK, 1], F32)
        nc.sync.dma_start(out=bz_sb[:], in_=bz_d[:])
        on_sb = consts.tile([NTK, E], MDT)
        nc.sync.dma_start(out=on_sb[:], in_=on_d[:])
        pt_sb = consts.tile([NTK, E], MDT)
        nc.sync.dma_start(out=pt_sb[:], in_=pt_d[:])
        wt_sb = consts.tile([E, 9 * E], MDT)
        for t in range(9):
            nc.sync.dma_start(out=wt_sb[:, t * E:(t + 1) * E], in_=wt_d[t])

        n_strips = h // r_out
        PPS = r_out // 2   # conv pairs per strip
        conv_queue = deque()
        cur_out = {"t": None}
        last_exp_inst = None

        def emit_conv_pair(item):
            istt, yA, drain_sel = item
            cP3, cr0 = istt["P3"], istt["r0"]
            pcv = ppool.tile([128, 512], F32, tag="bank")
            taps = []
            for ky in (1, 0, 2):
                rlo, rhi = yA, yA + 1
                if yA + ky - 1 < 0:
                    rlo = yA + 1
                if yA + 1 + ky - 1 > h - 1:
                    rhi = yA
                for kx in (0, 1, 2):
                    taps.append((ky, kx, rlo, rhi))
            for ti, (ky, kx, rlo, rhi) in enumerate(taps):
                nr = rhi - rlo + 1
                lr = rlo + ky - 1 - cr0
                tap = ky * 3 + kx
                nc.tensor.matmul(
                    pcv[:, (rlo - yA) * w:(rhi - yA + 1) * w],
                    wt_sb[:, tap * E:(tap + 1) * E],
                    cP3[:, lr:lr + nr, kx:kx + w],
                    start=(ti == 0), stop=(ti == len(taps) - 1))
            # drain into the 8-row staging buffer; DMA when full
            q = (yA // 2) % 4
            if q == 0:
                cur_out["t"] = pout.tile([128, 2048], BF16, tag="outsb",
                                         name="outsb")
            dst = cur_out["t"][:, q * 512:(q + 1) * 512]
            if drain_sel == 0:
                nc.vector.tensor_copy(dst, pcv[:])
            else:
                nc.scalar.copy(dst, pcv[:])
            if q == 3:
                g0 = yA - 6
                nc.gpsimd.dma_start(out=out_d[:, g0 * w:(g0 + 8) * w],
                                      in_=cur_out["t"][:])

        def emit_body():
            nonlocal last_exp_inst
            last_exp_inst = None
            for s in range(n_strips):
                stt = make_strip(s)
                emit_phaseA(stt)
                emit_unified(stt, 0, stt["NCH"] + 3)
            while conv_queue:
                emit_conv_pair(conv_queue.popleft())

        def make_strip(s):
            y0, y1 = s * r_out, (s + 1) * r_out
            r0, r1 = max(0, y0 - 1), min(h - 1, y1)
            chunks = []
            r = r0
            while r <= r1:
                nrows = min(2, r1 - r + 1)
                chunks.append((r, nrows))
                r += nrows
            return {"s": s, "y0": y0, "y1": y1, "r0": r0, "r1": r1,
                    "ir": r1 - r0 + 1, "chunks": chunks, "NCH": len(chunks),
                    "P3": None, "g": [], "wun": {}, "rb": {}, "ppu": {},
                    "s3n": 0, "s0done": 0}

        def pair_ready(item):
            stt, yA, dsel = item
            p = (yA - stt["y0"]) // 2
            return stt["s3n"] >= min(p + 3, stt["NCH"])

        def emit_phaseA(stt):
            nonlocal last_exp_inst
            s = stt["s"]
            r0, r1 = stt["r0"], stt["r1"]
            ir = stt["ir"]
            spix = ir * w
            off = r0 * w
            # strip-level input load (double-buffered via pool bufs=2);
            # [128, 2, spix] fp8 with k-tile dim for the DoubleRow matmul
            xfb_t = pin.tile([128, 2 * SW], F8, tag="xf")
            xfb3 = xfb_t[:].rearrange("p (t c) -> p t c", t=2)
            nc.sync.dma_start(out=xfb3[:, :, :spix],
                              in_=xfb_d[:, :, off:off + spix])
            stt["xfb3"] = xfb3

            P_t = pP.tile([128, IR_MAX * PITCH], MDT, tag="P")
            P3 = P_t[:].rearrange("p (r c) -> p r c", c=PITCH)
            stt["P3"] = P3
            # zero spacer columns (left/right conv zero-padding)
            nc.vector.memset(P3[:, :ir, 0:1], 0.0)
            nc.vector.memset(P3[:, :ir, PITCH - 1:PITCH], 0.0)
            for pi in range((stt["y1"] - stt["y0"]) // 2):
                conv_queue.append((stt, stt["y0"] + 2 * pi, pi % 2))
            first_gelu_inst = None
            for (r, nrows) in stt["chunks"]:
                npix = nrows * w
                loff = (r - r0) * w
                pg = ppool.tile([16, 512], F32, tag="bank", name="pg")
                nc.tensor.matmul(
                    pg[:, :npix],
                    wab_sb[:].rearrange("p (t m) -> p t m", t=2),
                    xfb3[:, :, loff:loff + npix],
                    start=True, stop=True,
                    perf_mode=mybir.MatmulPerfMode.DoubleRow)
                g_t = pg_pool.tile([14, 512], MDT, tag="g")
                inst = nc.scalar.activation(g_t[:, :npix], pg[0:14, :npix],
                                            AFT.Gelu, bias=b1_sb[:])
                if first_gelu_inst is None:
                    first_gelu_inst = inst
                stt["g"].append(g_t)
                # keep ~5 ready pairs in reserve to cover the exp-table-load
                # bubble at the phaseA->unified boundary (PPS entries at the
                # queue tail are this strip's own, not yet ready)
                if (len(conv_queue) > PPS + 5
                        and pair_ready(conv_queue[0])):
                    emit_conv_pair(conv_queue.popleft())
            # keep ACT phases ordered across strips so walrus doesn't reload
            # activation tables on interleaved gelu/exp runs
            if (last_exp_inst is not None and first_gelu_inst is not None
                    and ACT_DEP):
                bass._add_dep_helper(first_gelu_inst.ins, last_exp_inst.ins,
                                     sync=True, reason="act-table-phase-order")

        def stage0(stt, ci):   # fc2+expand matmul, exp
            nonlocal last_exp_inst
            r, nrows = stt["chunks"][ci]
            npix = nrows * w
            pzs = ppool.tile([NTK, 512], F32, tag="bank", name="pzs")
            nc.tensor.matmul(pzs[0:NTK, :npix], mz_sb[:],
                             stt["g"][ci][:, :npix])
            wun = psb.tile([NTK, 512], MDT, tag="wun", name="wun")
            stt["wun"][ci] = wun
            last_exp_inst = nc.scalar.activation(
                wun[:, :npix], pzs[0:NTK, :npix], AFT.Exp, bias=bz_sb[:])
            stt["s0done"] = ci + 1

        def stage1(stt, ci):   # S broadcast matmul + recip; prompt matmul
            r, nrows = stt["chunks"][ci]
            npix = nrows * w
            sb_p = ppool.tile([128, 512], F32, tag="bank", name="sb")
            nc.tensor.matmul(sb_p[:, :npix], on_sb[:],
                             stt["wun"][ci][:, :npix])
            ppu = ppool.tile([128, 512], F32, tag="bank", name="ppu")
            stt["ppu"][ci] = ppu
            nc.tensor.matmul(ppu[:, :npix], pt_sb[:],
                             stt["wun"][ci][:, :npix])
            rb = psb.tile([128, 512], F32, tag="rb", name="rb")
            stt["rb"][ci] = rb
            nc.vector.reciprocal_approx_fast(rb[:, :npix], sb_p[:, :npix])

        def stage2(stt, ci):   # fused normalize + copy into P strip
            r, nrows = stt["chunks"][ci]
            npix = nrows * w
            lr = r - stt["r0"]
            dst = stt["P3"][:, lr:lr + nrows, 1:1 + w]
            u3 = stt["ppu"][ci][:, :npix].rearrange("p (r c) -> p r c", c=w)
            rb3 = stt["rb"][ci][:, :npix].rearrange("p (r c) -> p r c", c=w)
            nc.vector.scalar_tensor_tensor(
                out=dst, in0=u3, scalar=1.0, in1=rb3,
                op0=mybir.AluOpType.mult, op1=mybir.AluOpType.mult)

        def emit_unified(stt, it0, it1):
            NCH = stt["NCH"]
            for it in range(it0, it1):
                if it < NCH and it >= stt["s0done"]:
                    stage0(stt, it)
                if it == it0:
                    # front-load fillers into the PE stream before stage1(0)
                    # emits the first exp-dependent matmul: they execute
                    # during the ACT table-load + exp(0) bubble
                    for _ in range(3):
                        if conv_queue and pair_ready(conv_queue[0]):
                            emit_conv_pair(conv_queue.popleft())
                if 0 <= it - 1 < NCH:
                    stage1(stt, it - 1)
                if 0 <= it - 2 < NCH:
                    stage2(stt, it - 2)
                    stt["s3n"] = it - 1
                while conv_queue:
                    if not pair_ready(conv_queue[0]):
                        break
                    hold = PPS if stt["s"] > 0 else 4
                    if (len(conv_queue) <= hold
                            and stt["s"] < n_strips - 1):
                        break  # keep filler pairs for next strip's phase A
                    emit_conv_pair(conv_queue.popleft())
                    break  # at most one pair per iteration

        emit_body()
        if not external_io:
            nc.sync.dma_start(out=outs_d[:], in_=out_d[0:1, 0:8])

    nc.compile()
    return nc


_cache = {}


def _bf16():
    import ml_dtypes
    return ml_dtypes.bfloat16


def get_program(h=256, w=256, r_out=64):
    key = (h, w, r_out)
    if key not in _cache:
        _cache[key] = build_program(h, w, r_out)
    return _cache[key]


def _fp8():
    import ml_dtypes
    return ml_dtypes.float8_e4m3


def make_weight_inputs(prompt, conv_w, b_fc1_w, b_fc1_b, b_fc2_w, b_fc2_b,
                       t_fc1_w, t_fc1_b, t_fc2_w, t_fc2_b):
    f = np.float32
    bf = _bf16()
    wa = np.zeros((128, 14), f)
    wa[:64, :8] = b_fc1_w.T
    wa[64:128, 8:14] = t_fc1_w[:, :64].T
    wb = np.zeros((128, 14), f)
    wb[:64, 8:14] = t_fc1_w[:, 64:].T
    wab = np.zeros((128, 2, 16), f)
    wab[:, 0, :14] = wa
    wab[:, 1, :14] = wb
    wab = wab.reshape(128, 32)
    b1 = np.zeros((14, 1), f)
    b1[:8, 0] = b_fc1_b
    b1[8:14, 0] = t_fc1_b
    mz = np.zeros((14, NTK), f)
    bz = np.zeros((NTK, 1), f)
    for t in range(NT):
        for k in range(NB):
            c = t * NB + k
            mz[:8, c] = b_fc2_w[k, :]
            mz[8:, c] = t_fc2_w[t, :]
            bz[c, 0] = b_fc2_b[k] + t_fc2_b[t]
    return {
        "wab": wab.astype(_fp8()),
        "b1": b1,
        "mz": mz.astype(bf),
        "bz": bz,
        "on": np.ones((NTK, E), bf),
        "pt": np.ascontiguousarray(prompt.reshape(NTK, E)).astype(bf),
        "wt": np.ascontiguousarray(
            conv_w.transpose(2, 3, 1, 0).reshape(9, E, E)).astype(bf),
    }


def make_core_inputs(x_b, flux_b, weights, h, w):
    PIX = h * w
    xfb = np.zeros((128, 2, PIX), np.float32)
    xfb[:DIM, 0] = x_b.reshape(DIM, PIX)
    xfb[DIM:, 0] = flux_b[:64].reshape(64, PIX)
    xfb[:64, 1] = flux_b[64:].reshape(64, PIX)
    m = {"xfb": xfb.astype(_fp8())}
    m.update(weights)
    return m


def kernel(x, flux, prompt, conv_w, b_fc1_w, b_fc1_b, b_fc2_w, b_fc2_b,
           t_fc1_w, t_fc1_b, t_fc2_w, t_fc2_b):
    x = np.asarray(x, np.float32)
    flux = np.asarray(flux, np.float32)
    flux = np.where(np.isnan(flux), np.float32(0), flux)
    h, w = x.shape[2], x.shape[3]

    nc = get_program(h=h, w=w)
    weights = make_weight_inputs(
        np.asarray(prompt, np.float32), np.asarray(conv_w, np.float32),
        np.asarray(b_fc1_w, np.float32), np.asarray(b_fc1_b, np.float32),
        np.asarray(b_fc2_w, np.float32), np.asarray(b_fc2_b, np.float32),
        np.asarray(t_fc1_w, np.float32), np.asarray(t_fc1_b, np.float32),
        np.asarray(t_fc2_w, np.float32), np.asarray(t_fc2_b, np.float32))
    in_maps = [make_core_inputs(x[i], flux[i], weights, h, w)
               for i in range(NCORES)]
    res = run_bass_kernel_spmd(nc, in_maps, list(range(NCORES)))
    out = np.stack([res.results[i]["out"].astype(np.float32).reshape(E, h, w)
                    for i in range(NCORES)], axis=0)
    return out


# revision 40
# speedup vs baseline: 1.0256x; 1.0256x over previous
"""Trainium2 Bass kernel: ActiveBlockPromptBasis (moe_routing).

Math (per batch image b):
  g   = gelu(W1x @ x_b + W1t @ flux_b + b1)                    # [14, pix]
  z   = Mz.T @ g + bz          (fc2 of both MLPs fused with the 6x8
                                outer-sum expansion, in log space)  # [48, pix]
  wun = exp(z)                                                 # exact softmax
  S   = colsum(wun)  broadcast to 128 partitions in ONE matmul
        (lhsT = ones[48,128]  ->  psum[p, n] = S[n] for all p)
  U   = prompt_flat.T @ wun                                    # [128, pix]
  P   = U * recip(S_bcast)     (DVE approx-recip [128,n] + fused
                                normalize-multiply into the P strip)
  out = conv3x3(P, conv_w)     (9 accumulating bf16 matmuls / PSUM bank;
                                zero spacer columns implement padding)

Implementation notes:
  - All matmul operands are bf16 (fp32 PSUM accumulate).  End-to-end
    relative error vs the fp32 reference is ~2e-3; output is returned as
    bf16 and widened on the host.
  - The ones[48,128] stationary operand makes the column-sum land
    broadcast across all 128 partitions directly, so no DRAM-bounce
    broadcast DMA is needed (the old version moved ~40MB through HBM
    for 1/S replication).
  - Inputs are loaded one STRIP at a time (2 big DMAs per strip,
    double-buffered) instead of per-chunk, cutting DMA issuance ~17x.
  - Engines issue in order, so the per-chunk softmax relay chain is
    software-pipelined at emission: stage k of chunk i is emitted next
    to stage k+1 of chunk i-1, with always-ready conv matmul pairs
    interleaved as PE gap filler.
  - Gelu and Exp live in different ACT table sets, so each strip runs a
    gelu-only phase then an exp-only phase, with an explicit cross-strip
    ACT ordering dep.

Sharding: data-parallel over batch, one image per NeuronCore (8 cores).
"""

import numpy as np
from collections import deque
from contextlib import ExitStack

import concourse.bass as bass
import concourse.tile as tile
from concourse import bacc, mybir
from concourse.bass_utils import run_bass_kernel_spmd

F32 = mybir.dt.float32
BF16 = mybir.dt.bfloat16
AFT = mybir.ActivationFunctionType

B, DIM, E = 8, 64, 128
NT, NB = 6, 8
NTK = NT * NB  # 48
NCORES = 8


def build_program(h=256, w=256, r_out=32, external_io=True):
    """Build the single-core Bass program (SPMD: same program on all cores)."""
    assert w == 256 and r_out % 8 == 0 and h % r_out == 0
    PIX = h * w

    MDT = BF16
    ACT_DEP = True
    PITCH = w + 2          # row window with 1 zero spacer col each side
    IR_MAX = r_out + 2     # input rows per strip incl halo
    SW = (r_out + 2) * w   # strip pixels incl halo (upper bound)

    nc = bacc.Bacc("TRN2", target_bir_lowering=False, debug=False,
                   enable_asserts=False)

    # --- DRAM I/O (per-core slices / replicated small weights) ---
    kin = "ExternalInput" if external_io else "Internal"
    kout = "ExternalOutput" if external_io else "Internal"
    F8 = mybir.dt.float8e4
    xfb_d = nc.dram_tensor("xfb", [128, 2, PIX], F8, kind=kin)
    wab_d = nc.dram_tensor("wab", [128, 2 * 16], F8, kind=kin)
    b1_d = nc.dram_tensor("b1", [14, 1], F32, kind=kin)
    mz_d = nc.dram_tensor("mz", [14, NTK], MDT, kind=kin)
    bz_d = nc.dram_tensor("bz", [NTK, 1], F32, kind=kin)
    on_d = nc.dram_tensor("on", [NTK, E], MDT, kind=kin)
    pt_d = nc.dram_tensor("pt", [NTK, E], MDT, kind=kin)
    wt_d = nc.dram_tensor("wt", [9, E, E], MDT, kind=kin)
    out_d = nc.dram_tensor("out", [E, PIX], BF16, kind=kout)
    if not external_io:
        outs_d = nc.dram_tensor("outs", [1, 8], BF16, kind="ExternalOutput")

    with tile.TileContext(nc) as tc, ExitStack() as ctx:
        consts = ctx.enter_context(tc.tile_pool(name="consts", bufs=1))
        pin = ctx.enter_context(tc.tile_pool(name="pin", bufs=2))
        psb = ctx.enter_context(tc.tile_pool(name="psb", bufs=6))
        pg_pool = ctx.enter_context(tc.tile_pool(name="pg", bufs=IR_MAX // 2 + 2))
        ppool = ctx.enter_context(
            tc.tile_pool(name="ppsum", bufs=8, space="PSUM"))
        pP = ctx.enter_context(tc.tile_pool(name="pP", bufs=2))
        pout = ctx.enter_context(tc.tile_pool(name="pout", bufs=2))

        # --- load constants once ---
        wab_sb = consts.tile([128, 2 * 16], F8)
        nc.sync.dma_start(out=wab_sb[:], in_=wab_d[:])
        b1_sb = consts.tile([14, 1], F32)
        nc.sync.dma_start(out=b1_sb[:], in_=b1_d[:])
        mz_sb = consts.tile([14, NTK], MDT)
        nc.sync.dma_start(out=mz_sb[:], in_=mz_d[:])
        bz_sb = consts.tile([NTK, 1], F32)
        nc.sync.dma_start(out=bz_sb[:], in_=bz_d[:])
        on_sb = consts.tile([NTK, E], MDT)
        nc.sync.dma_start(out=on_sb[:], in_=on_d[:])
        pt_sb = consts.tile([NTK, E], MDT)
        nc.sync.dma_start(out=pt_sb[:], in_=pt_d[:])
        wt_sb = consts.tile([E, 9 * E], MDT)
        for t in range(9):
            nc.sync.dma_start(out=wt_sb[:, t * E:(t + 1) * E], in_=wt_d[t])

        n_strips = h // r_out
        PPS = r_out // 2   # conv pairs per strip
        conv_queue = deque()
        cur_out = {"t": None}
        last_exp_inst = None

        def emit_conv_pair(item):
            istt, yA, drain_sel = item
            cP3, cr0 = istt["P3"], istt["r0"]
            pcv = ppool.tile([128, 512], F32, tag="bank")
            taps = []
            for ky in (1, 0, 2):
                rlo, rhi = yA, yA + 1
                if yA + ky - 1 < 0:
                    rlo = yA + 1
                if yA + 1 + ky - 1 > h - 1:
                    rhi = yA
                for kx in (0, 1, 2):
                    taps.append((ky, kx, rlo, rhi))
            for ti, (ky, kx, rlo, rhi) in enumerate(taps):
                nr = rhi - rlo + 1
                lr = rlo + ky - 1 - cr0
                tap = ky * 3 + kx
                nc.tensor.matmul(
                    pcv[:, (rlo - yA) * w:(rhi - yA + 1) * w],
                    wt_sb[:, tap * E:(tap + 1) * E],
                    cP3[:, lr:lr + nr, kx:kx + w],
                    start=(ti == 0), stop=(ti == len(taps) - 1))
            # drain into the 8-row staging buffer; DMA when full
            q = (yA // 2) % 4
            if q == 0:
                cur_out["t"] = pout.tile([128, 2048], BF16, tag="outsb",
                                         name="outsb")
            dst = cur_out["t"][:, q * 512:(q + 1) * 512]
            if drain_sel == 0:
                nc.vector.tensor_copy(dst, pcv[:])
            else:
                nc.scalar.copy(dst, pcv[:])
            if q == 3:
                g0 = yA - 6
                nc.gpsimd.dma_start(out=out_d[:, g0 * w:(g0 + 8) * w],
                                      in_=cur_out["t"][:])

        def emit_body():
            nonlocal last_exp_inst
            last_exp_inst = None
            for s in range(n_strips):
                stt = make_strip(s)
                emit_phaseA(stt)
                emit_unified(stt, 0, stt["NCH"] + 3)
            while conv_queue:
                emit_conv_pair(conv_queue.popleft())

        def make_strip(s):
            y0, y1 = s * r_out, (s + 1) * r_out
            r0, r1 = max(0, y0 - 1), min(h - 1, y1)
            chunks = []
            r = r0
            while r <= r1:
                nrows = min(2, r1 - r + 1)
                chunks.append((r, nrows))
                r += nrows
            return {"s": s, "y0": y0, "y1": y1, "r0": r0, "r1": r1,
                    "ir": r1 - r0 + 1, "chunks": chunks, "NCH": len(chunks),
                    "P3": None, "g": [], "wun": {}, "rb": {}, "ppu": {},
                    "s3n": 0, "s0done": 0}

        def pair_ready(item):
            stt, yA, dsel = item
            p = (yA - stt["y0"]) // 2
            return stt["s3n"] >= min(p + 3, stt["NCH"])

        def emit_phaseA(stt):
            nonlocal last_exp_inst
            s = stt["s"]
            r0, r1 = stt["r0"], stt["r1"]
            ir = stt["ir"]
            spix = ir * w
            off = r0 * w
            # strip-level input load (double-buffered via pool bufs=2);
            # [128, 2, spix] fp8 with k-tile dim for the DoubleRow matmul
            xfb_t = pin.tile([128, 2 * SW], F8, tag="xf")
            xfb3 = xfb_t[:].rearrange("p (t c) -> p t c", t=2)
            if s == 0:
                # per-chunk loads so the first matmul starts after ~128KB
                # instead of the whole strip
                co = 0
                for (r, nrows) in stt["chunks"]:
                    npx = nrows * w
                    eng = nc.sync if (co // 512) % 2 == 0 else nc.scalar
                    eng.dma_start(out=xfb3[:, :, co:co + npx],
                                  in_=xfb_d[:, :, off + co:off + co + npx])
                    co += npx
            else:
                nc.sync.dma_start(out=xfb3[:, :, :spix],
                                  in_=xfb_d[:, :, off:off + spix])
            stt["xfb3"] = xfb3

            P_t = pP.tile([128, IR_MAX * PITCH], MDT, tag="P")
            P3 = P_t[:].rearrange("p (r c) -> p r c", c=PITCH)
            stt["P3"] = P3
            # zero spacer columns (left/right conv zero-padding)
            nc.vector.memset(P3[:, :ir, 0:1], 0.0)
            nc.vector.memset(P3[:, :ir, PITCH - 1:PITCH], 0.0)
            for pi in range((stt["y1"] - stt["y0"]) // 2):
                conv_queue.append((stt, stt["y0"] + 2 * pi, pi % 2))
            first_gelu_inst = None
            for (r, nrows) in stt["chunks"]:
                npix = nrows * w
                loff = (r - r0) * w
                pg = ppool.tile([16, 512], F32, tag="bank", name="pg")
                nc.tensor.matmul(
                    pg[:, :npix],
                    wab_sb[:].rearrange("p (t m) -> p t m", t=2),
                    xfb3[:, :, loff:loff + npix],
                    start=True, stop=True,
                    perf_mode=mybir.MatmulPerfMode.DoubleRow)
                g_t = pg_pool.tile([14, 512], MDT, tag="g")
                inst = nc.scalar.activation(g_t[:, :npix], pg[0:14, :npix],
                                            AFT.Gelu, bias=b1_sb[:])
                if first_gelu_inst is None:
                    first_gelu_inst = inst
                stt["g"].append(g_t)
                # keep ~5 ready pairs in reserve to cover the exp-table-load
                # bubble at the phaseA->unified boundary (PPS entries at the
                # queue tail are this strip's own, not yet ready)
                if (len(conv_queue) > PPS + 5
                        and pair_ready(conv_queue[0])):
                    emit_conv_pair(conv_queue.popleft())
            # keep ACT phases ordered across strips so walrus doesn't reload
            # activation tables on interleaved gelu/exp runs
            if (last_exp_inst is not None and first_gelu_inst is not None
                    and ACT_DEP):
                bass._add_dep_helper(first_gelu_inst.ins, last_exp_inst.ins,
                                     sync=True, reason="act-table-phase-order")

        def stage0(stt, ci):   # fc2+expand matmul, exp
            nonlocal last_exp_inst
            r, nrows = stt["chunks"][ci]
            npix = nrows * w
            pzs = ppool.tile([NTK, 512], F32, tag="bank", name="pzs")
            nc.tensor.matmul(pzs[0:NTK, :npix], mz_sb[:],
                             stt["g"][ci][:, :npix])
            wun = psb.tile([NTK, 512], MDT, tag="wun", name="wun")
            stt["wun"][ci] = wun
            last_exp_inst = nc.scalar.activation(
                wun[:, :npix], pzs[0:NTK, :npix], AFT.Exp, bias=bz_sb[:])
            stt["s0done"] = ci + 1

        def stage1(stt, ci):   # S broadcast matmul + recip; prompt matmul
            r, nrows = stt["chunks"][ci]
            npix = nrows * w
            sb_p = ppool.tile([128, 512], F32, tag="bank", name="sb")
            nc.tensor.matmul(sb_p[:, :npix], on_sb[:],
                             stt["wun"][ci][:, :npix])
            ppu = ppool.tile([128, 512], F32, tag="bank", name="ppu")
            stt["ppu"][ci] = ppu
            nc.tensor.matmul(ppu[:, :npix], pt_sb[:],
                             stt["wun"][ci][:, :npix])
            rb = psb.tile([128, 512], F32, tag="rb", name="rb")
            stt["rb"][ci] = rb
            nc.vector.reciprocal_approx_fast(rb[:, :npix], sb_p[:, :npix])

        def stage2(stt, ci):   # fused normalize + copy into P strip
            r, nrows = stt["chunks"][ci]
            npix = nrows * w
            lr = r - stt["r0"]
            dst = stt["P3"][:, lr:lr + nrows, 1:1 + w]
            u3 = stt["ppu"][ci][:, :npix].rearrange("p (r c) -> p r c", c=w)
            rb3 = stt["rb"][ci][:, :npix].rearrange("p (r c) -> p r c", c=w)
            nc.vector.scalar_tensor_tensor(
                out=dst, in0=u3, scalar=1.0, in1=rb3,
                op0=mybir.AluOpType.mult, op1=mybir.AluOpType.mult)

        def emit_unified(stt, it0, it1):
            NCH = stt["NCH"]
            for it in range(it0, it1):
                if it < NCH and it >= stt["s0done"]:
                    stage0(stt, it)
                if it == it0:
                    # front-load fillers into the PE stream before stage1(0)
                    # emits the first exp-dependent matmul: they execute
                    # during the ACT table-load + exp(0) bubble
                    for _ in range(3):
                        if conv_queue and pair_ready(conv_queue[0]):
                            emit_conv_pair(conv_queue.popleft())
                if 0 <= it - 1 < NCH:
                    stage1(stt, it - 1)
                if 0 <= it - 2 < NCH:
                    stage2(stt, it - 2)
                    stt["s3n"] = it - 1
                while conv_queue:
                    if not pair_ready(conv_queue[0]):
                        break
                    hold = PPS if stt["s"] > 0 else 4
                    if (len(conv_queue) <= hold
                            and stt["s"] < n_strips - 1):
                        break  # keep filler pairs for next strip's phase A
                    emit_conv_pair(conv_queue.popleft())
                    break  # at most one pair per iteration

        emit_body()
        if not external_io:
            nc.sync.dma_start(out=outs_d[:], in_=out_d[0:1, 0:8])

    nc.compile()
    return nc


_cache = {}


def _bf16():
    import ml_dtypes
    return ml_dtypes.bfloat16


def get_program(h=256, w=256, r_out=64):
    key = (h, w, r_out)
    if key not in _cache:
        _cache[key] = build_program(h, w, r_out)
    return _cache[key]


def _fp8():
    import ml_dtypes
    return ml_dtypes.float8_e4m3


def make_weight_inputs(prompt, conv_w, b_fc1_w, b_fc1_b, b_fc2_w, b_fc2_b,
                       t_fc1_w, t_fc1_b, t_fc2_w, t_fc2_b):
    f = np.float32
    bf = _bf16()
    wa = np.zeros((128, 14), f)
    wa[:64, :8] = b_fc1_w.T
    wa[64:128, 8:14] = t_fc1_w[:, :64].T
    wb = np.zeros((128, 14), f)
    wb[:64, 8:14] = t_fc1_w[:, 64:].T
    wab = np.zeros((128, 2, 16), f)
    wab[:, 0, :14] = wa
    wab[:, 1, :14] = wb
    wab = wab.reshape(128, 32)
    b1 = np.zeros((14, 1), f)
    b1[:8, 0] = b_fc1_b
    b1[8:14, 0] = t_fc1_b
    mz = np.zeros((14, NTK), f)
    bz = np.zeros((NTK, 1), f)
    for t in range(NT):
        for k in range(NB):
            c = t * NB + k
            mz[:8, c] = b_fc2_w[k, :]
            mz[8:, c] = t_fc2_w[t, :]
            bz[c, 0] = b_fc2_b[k] + t_fc2_b[t]
    return {
        "wab": wab.astype(_fp8()),
        "b1": b1,
        "mz": mz.astype(bf),
        "bz": bz,
        "on": np.ones((NTK, E), bf),
        "pt": np.ascontiguousarray(prompt.reshape(NTK, E)).astype(bf),
        "wt": np.ascontiguousarray(
            conv_w.transpose(2, 3, 1, 0).reshape(9, E, E)).astype(bf),
    }


def make_core_inputs(x_b, flux_b, weights, h, w):
    PIX = h * w
    xfb = np.zeros((128, 2, PIX), np.float32)
    xfb[:DIM, 0] = x_b.reshape(DIM, PIX)
    xfb[DIM:, 0] = flux_b[:64].reshape(64, PIX)
    xfb[:64, 1] = flux_b[64:].reshape(64, PIX)
    m = {"xfb": xfb.astype(_fp8())}
    m.update(weights)
    return m


def kernel(x, flux, prompt, conv_w, b_fc1_w, b_fc1_b, b_fc2_w, b_fc2_b,
           t_fc1_w, t_fc1_b, t_fc2_w, t_fc2_b):
    x = np.asarray(x, np.float32)
    flux = np.asarray(flux, np.float32)
    flux = np.where(np.isnan(flux), np.float32(0), flux)
    h, w = x.shape[2], x.shape[3]

    nc = get_program(h=h, w=w)
    weights = make_weight_inputs(
        np.asarray(prompt, np.float32), np.asarray(conv_w, np.float32),
        np.asarray(b_fc1_w, np.float32), np.asarray(b_fc1_b, np.float32),
        np.asarray(b_fc2_w, np.float32), np.asarray(b_fc2_b, np.float32),
        np.asarray(t_fc1_w, np.float32), np.asarray(t_fc1_b, np.float32),
        np.asarray(t_fc2_w, np.float32), np.asarray(t_fc2_b, np.float32))
    in_maps = [make_core_inputs(x[i], flux[i], weights, h, w)
               for i in range(NCORES)]
    res = run_bass_kernel_spmd(nc, in_maps, list(range(NCORES)))
    out = np.stack([res.results[i]["out"].astype(np.float32).reshape(E, h, w)
                    for i in range(NCORES)], axis=0)
    return out


# revision 41
# speedup vs baseline: 1.0345x; 1.0087x over previous
"""Trainium2 Bass kernel: ActiveBlockPromptBasis (moe_routing).

Math (per batch image b):
  g   = gelu(Wab.T @ [x;flux] + b1)   one fp8 DoubleRow matmul (K=192
                                      packed as 2 k-tiles of 128)    # [14, pix]
  z   = Mz.T @ g + bz          (fc2 of both MLPs fused with the 6x8
                                outer-sum expansion, in log space)  # [48, pix]
  wun = exp(z)                                                 # exact softmax
  S   = colsum(wun)  broadcast to 128 partitions in ONE matmul
        (lhsT = ones[48,128]  ->  psum[p, n] = S[n] for all p)
  U   = prompt_flat.T @ wun                                    # [128, pix]
  P   = U * recip(S_bcast)     (DVE approx-recip [128,n] + fused
                                normalize-multiply into the P strip)
  out = conv3x3(P, conv_w)     (9 accumulating bf16 matmuls / PSUM bank;
                                zero spacer columns implement padding)

Implementation notes (vs the f32r version this replaces, ~570us -> ~520us):
  - All matmul operands are bf16 (fp32 PSUM accumulate) except the first
    layer, which runs the x/flux contraction (K=192) in a single fp8e4m3
    DoubleRow matmul (weight cols padded to 16 to satisfy the dual-fp8
    ldweights stride rule).  End-to-end rel err vs the fp32 reference is
    ~6e-3 (threshold 2e-2); output returns as bf16, widened on host.
  - The ones[48,128] stationary operand makes the column-sum land
    broadcast across all 128 partitions directly, so no DRAM-bounce
    broadcast DMA is needed (the old version moved ~40MB through HBM for
    1/S replication; HBM traffic is now 30MB/core: 17 in fp8, 17 out bf16).
  - Inputs are loaded one STRIP at a time (one big DMA, double-buffered)
    instead of per-chunk; strip 0 loads per-chunk so the PE starts early.
  - Engines issue in order, so the per-chunk softmax relay chain is
    software-pipelined at emission: stage k of chunk i is emitted next to
    stage k+1 of chunk i-1, with always-ready conv matmul pairs
    interleaved as PE gap filler.  A reserve of ready conv pairs is kept
    back and 3 are front-loaded right after each strip's first fc2
    matmul, so the PE stays busy across the ACT exp-table-load bubble at
    every strip boundary (this also stops the HAM clock-gate from
    rethrottling the PE to 1.2GHz there).
  - Gelu and Exp live in different ACT table sets, so each strip runs a
    gelu-only phase then an exp-only phase, with an explicit cross-strip
    ACT ordering dep.  r_out=64 (8 strips) halves the table reloads.
  - Conv PSUM drains alternate DVE/ACT (GPSIMD cannot read PSUM); output
    DMA rides SWDGE on gpsimd, inputs on the sync/scalar HWDGE queues.
  - Measured (NTFF, core 0): PE busy ~87% of span, ~139ns per 512-col
    matmul back-to-back (production roofline); residual cold time is the
    chip power throttle (K=4/8), not idle-driven.

Sharding: data-parallel over batch, one image per NeuronCore (8 cores).
"""

import numpy as np
from collections import deque
from contextlib import ExitStack

import concourse.bass as bass
import concourse.tile as tile
from concourse import bacc, mybir
from concourse.bass_utils import run_bass_kernel_spmd

F32 = mybir.dt.float32
BF16 = mybir.dt.bfloat16
AFT = mybir.ActivationFunctionType

B, DIM, E = 8, 64, 128
NT, NB = 6, 8
NTK = NT * NB  # 48
NCORES = 8


def build_program(h=256, w=256, r_out=32, external_io=True):
    """Build the single-core Bass program (SPMD: same program on all cores)."""
    assert w == 256 and r_out % 8 == 0 and h % r_out == 0
    PIX = h * w

    MDT = BF16
    ACT_DEP = True
    PITCH = w + 2          # row window with 1 zero spacer col each side
    IR_MAX = r_out + 2     # input rows per strip incl halo
    SW = (r_out + 2) * w   # strip pixels incl halo (upper bound)

    nc = bacc.Bacc("TRN2", target_bir_lowering=False, debug=False,
                   enable_asserts=False)

    # --- DRAM I/O (per-core slices / replicated small weights) ---
    kin = "ExternalInput" if external_io else "Internal"
    kout = "ExternalOutput" if external_io else "Internal"
    F8 = mybir.dt.float8e4
    xfb_d = nc.dram_tensor("xfb", [128, 2, PIX], F8, kind=kin)
    wab_d = nc.dram_tensor("wab", [128, 2 * 16], F8, kind=kin)
    b1_d = nc.dram_tensor("b1", [14, 1], F32, kind=kin)
    mz_d = nc.dram_tensor("mz", [14, NTK], MDT, kind=kin)
    bz_d = nc.dram_tensor("bz", [NTK, 1], F32, kind=kin)
    on_d = nc.dram_tensor("on", [NTK, E], MDT, kind=kin)
    pt_d = nc.dram_tensor("pt", [NTK, E], MDT, kind=kin)
    wt_d = nc.dram_tensor("wt", [9, E, E], MDT, kind=kin)
    out_d = nc.dram_tensor("out", [E, PIX], BF16, kind=kout)
    if not external_io:
        outs_d = nc.dram_tensor("outs", [1, 8], BF16, kind="ExternalOutput")

    with tile.TileContext(nc) as tc, ExitStack() as ctx:
        consts = ctx.enter_context(tc.tile_pool(name="consts", bufs=1))
        pin = ctx.enter_context(tc.tile_pool(name="pin", bufs=2))
        psb = ctx.enter_context(tc.tile_pool(name="psb", bufs=6))
        pg_pool = ctx.enter_context(tc.tile_pool(name="pg", bufs=IR_MAX // 2 + 2))
        ppool = ctx.enter_context(
            tc.tile_pool(name="ppsum", bufs=8, space="PSUM"))
        pP = ctx.enter_context(tc.tile_pool(name="pP", bufs=2))
        pout = ctx.enter_context(tc.tile_pool(name="pout", bufs=2))

        # --- load constants once ---
        wab_sb = consts.tile([128, 2 * 16], F8)
        nc.sync.dma_start(out=wab_sb[:], in_=wab_d[:])
        b1_sb = consts.tile([14, 1], F32)
        nc.sync.dma_start(out=b1_sb[:], in_=b1_d[:])
        mz_sb = consts.tile([14, NTK], MDT)
        nc.sync.dma_start(out=mz_sb[:], in_=mz_d[:])
        bz_sb = consts.tile([NTK, 1], F32)
        nc.sync.dma_start(out=bz_sb[:], in_=bz_d[:])
        on_sb = consts.tile([NTK, E], MDT)
        nc.sync.dma_start(out=on_sb[:], in_=on_d[:])
        pt_sb = consts.tile([NTK, E], MDT)
        nc.sync.dma_start(out=pt_sb[:], in_=pt_d[:])
        wt_sb = consts.tile([E, 9 * E], MDT)
        for t in range(9):
            nc.sync.dma_start(out=wt_sb[:, t * E:(t + 1) * E], in_=wt_d[t])

        n_strips = h // r_out
        PPS = r_out // 2   # conv pairs per strip
        conv_queue = deque()
        cur_out = {"t": None}
        last_exp_inst = None

        def emit_conv_pair(item):
            istt, yA, drain_sel = item
            cP3, cr0 = istt["P3"], istt["r0"]
            pcv = ppool.tile([128, 512], F32, tag="bank")
            taps = []
            for ky in (1, 0, 2):
                rlo, rhi = yA, yA + 1
                if yA + ky - 1 < 0:
                    rlo = yA + 1
                if yA + 1 + ky - 1 > h - 1:
                    rhi = yA
                for kx in (0, 1, 2):
                    taps.append((ky, kx, rlo, rhi))
            for ti, (ky, kx, rlo, rhi) in enumerate(taps):
                nr = rhi - rlo + 1
                lr = rlo + ky - 1 - cr0
                tap = ky * 3 + kx
                nc.tensor.matmul(
                    pcv[:, (rlo - yA) * w:(rhi - yA + 1) * w],
                    wt_sb[:, tap * E:(tap + 1) * E],
                    cP3[:, lr:lr + nr, kx:kx + w],
                    start=(ti == 0), stop=(ti == len(taps) - 1))
            # drain into the 8-row staging buffer; DMA when full
            q = (yA // 2) % 4
            if q == 0:
                cur_out["t"] = pout.tile([128, 2048], BF16, tag="outsb",
                                         name="outsb")
            dst = cur_out["t"][:, q * 512:(q + 1) * 512]
            if drain_sel == 0:
                nc.vector.tensor_copy(dst, pcv[:])
            else:
                nc.scalar.copy(dst, pcv[:])
            if q == 3:
                g0 = yA - 6
                nc.gpsimd.dma_start(out=out_d[:, g0 * w:(g0 + 8) * w],
                                      in_=cur_out["t"][:])

        def emit_body():
            nonlocal last_exp_inst
            last_exp_inst = None
            for s in range(n_strips):
                stt = make_strip(s)
                emit_phaseA(stt)
                emit_unified(stt, 0, stt["NCH"] + 3)
            while conv_queue:
                emit_conv_pair(conv_queue.popleft())

        def make_strip(s):
            y0, y1 = s * r_out, (s + 1) * r_out
            r0, r1 = max(0, y0 - 1), min(h - 1, y1)
            chunks = []
            r = r0
            while r <= r1:
                nrows = min(2, r1 - r + 1)
                chunks.append((r, nrows))
                r += nrows
            return {"s": s, "y0": y0, "y1": y1, "r0": r0, "r1": r1,
                    "ir": r1 - r0 + 1, "chunks": chunks, "NCH": len(chunks),
                    "P3": None, "g": [], "wun": {}, "rb": {}, "ppu": {},
                    "s3n": 0, "s0done": 0}

        def pair_ready(item):
            stt, yA, dsel = item
            p = (yA - stt["y0"]) // 2
            return stt["s3n"] >= min(p + 3, stt["NCH"])

        def emit_phaseA(stt):
            nonlocal last_exp_inst
            s = stt["s"]
            r0, r1 = stt["r0"], stt["r1"]
            ir = stt["ir"]
            spix = ir * w
            off = r0 * w
            # strip-level input load (double-buffered via pool bufs=2);
            # [128, 2, spix] fp8 with k-tile dim for the DoubleRow matmul
            xfb_t = pin.tile([128, 2 * SW], F8, tag="xf")
            xfb3 = xfb_t[:].rearrange("p (t c) -> p t c", t=2)
            if s == 0:
                # per-chunk loads so the first matmul starts after ~128KB
                # instead of the whole strip
                co = 0
                for (r, nrows) in stt["chunks"]:
                    npx = nrows * w
                    eng = nc.sync if (co // 512) % 2 == 0 else nc.scalar
                    eng.dma_start(out=xfb3[:, :, co:co + npx],
                                  in_=xfb_d[:, :, off + co:off + co + npx])
                    co += npx
            else:
                nc.sync.dma_start(out=xfb3[:, :, :spix],
                                  in_=xfb_d[:, :, off:off + spix])
            stt["xfb3"] = xfb3

            P_t = pP.tile([128, IR_MAX * PITCH], MDT, tag="P")
            P3 = P_t[:].rearrange("p (r c) -> p r c", c=PITCH)
            stt["P3"] = P3
            # zero spacer columns (left/right conv zero-padding)
            nc.vector.memset(P3[:, :ir, 0:1], 0.0)
            nc.vector.memset(P3[:, :ir, PITCH - 1:PITCH], 0.0)
            for pi in range((stt["y1"] - stt["y0"]) // 2):
                conv_queue.append((stt, stt["y0"] + 2 * pi, pi % 2))
            first_gelu_inst = None
            for (r, nrows) in stt["chunks"]:
                npix = nrows * w
                loff = (r - r0) * w
                pg = ppool.tile([16, 512], F32, tag="bank", name="pg")
                nc.tensor.matmul(
                    pg[:, :npix],
                    wab_sb[:].rearrange("p (t m) -> p t m", t=2),
                    xfb3[:, :, loff:loff + npix],
                    start=True, stop=True,
                    perf_mode=mybir.MatmulPerfMode.DoubleRow)
                g_t = pg_pool.tile([14, 512], MDT, tag="g")
                inst = nc.scalar.activation(g_t[:, :npix], pg[0:14, :npix],
                                            AFT.Gelu, bias=b1_sb[:])
                if first_gelu_inst is None:
                    first_gelu_inst = inst
                stt["g"].append(g_t)
                # keep ~5 ready pairs in reserve to cover the exp-table-load
                # bubble at the phaseA->unified boundary (PPS entries at the
                # queue tail are this strip's own, not yet ready)
                if (len(conv_queue) > PPS + 5
                        and pair_ready(conv_queue[0])):
                    emit_conv_pair(conv_queue.popleft())
            # keep ACT phases ordered across strips so walrus doesn't reload
            # activation tables on interleaved gelu/exp runs
            if (last_exp_inst is not None and first_gelu_inst is not None
                    and ACT_DEP):
                bass._add_dep_helper(first_gelu_inst.ins, last_exp_inst.ins,
                                     sync=True, reason="act-table-phase-order")

        def stage0(stt, ci):   # fc2+expand matmul, exp
            nonlocal last_exp_inst
            r, nrows = stt["chunks"][ci]
            npix = nrows * w
            pzs = ppool.tile([NTK, 512], F32, tag="bank", name="pzs")
            nc.tensor.matmul(pzs[0:NTK, :npix], mz_sb[:],
                             stt["g"][ci][:, :npix])
            wun = psb.tile([NTK, 512], MDT, tag="wun", name="wun")
            stt["wun"][ci] = wun
            last_exp_inst = nc.scalar.activation(
                wun[:, :npix], pzs[0:NTK, :npix], AFT.Exp, bias=bz_sb[:])
            stt["s0done"] = ci + 1

        def stage1(stt, ci):   # S broadcast matmul + recip; prompt matmul
            r, nrows = stt["chunks"][ci]
            npix = nrows * w
            sb_p = ppool.tile([128, 512], F32, tag="bank", name="sb")
            nc.tensor.matmul(sb_p[:, :npix], on_sb[:],
                             stt["wun"][ci][:, :npix])
            ppu = ppool.tile([128, 512], F32, tag="bank", name="ppu")
            stt["ppu"][ci] = ppu
            nc.tensor.matmul(ppu[:, :npix], pt_sb[:],
                             stt["wun"][ci][:, :npix])
            rb = psb.tile([128, 512], F32, tag="rb", name="rb")
            stt["rb"][ci] = rb
            nc.vector.reciprocal_approx_fast(rb[:, :npix], sb_p[:, :npix])

        def stage2(stt, ci):   # fused normalize + copy into P strip
            r, nrows = stt["chunks"][ci]
            npix = nrows * w
            lr = r - stt["r0"]
            dst = stt["P3"][:, lr:lr + nrows, 1:1 + w]
            u3 = stt["ppu"][ci][:, :npix].rearrange("p (r c) -> p r c", c=w)
            rb3 = stt["rb"][ci][:, :npix].rearrange("p (r c) -> p r c", c=w)
            nc.vector.scalar_tensor_tensor(
                out=dst, in0=u3, scalar=1.0, in1=rb3,
                op0=mybir.AluOpType.mult, op1=mybir.AluOpType.mult)

        def emit_unified(stt, it0, it1):
            NCH = stt["NCH"]
            for it in range(it0, it1):
                if it < NCH and it >= stt["s0done"]:
                    stage0(stt, it)
                if it == it0:
                    # front-load fillers into the PE stream before stage1(0)
                    # emits the first exp-dependent matmul: they execute
                    # during the ACT table-load + exp(0) bubble
                    for _ in range(3):
                        if conv_queue and pair_ready(conv_queue[0]):
                            emit_conv_pair(conv_queue.popleft())
                if 0 <= it - 1 < NCH:
                    stage1(stt, it - 1)
                if 0 <= it - 2 < NCH:
                    stage2(stt, it - 2)
                    stt["s3n"] = it - 1
                while conv_queue:
                    if not pair_ready(conv_queue[0]):
                        break
                    hold = PPS if stt["s"] > 0 else 4
                    if (len(conv_queue) <= hold
                            and stt["s"] < n_strips - 1):
                        break  # keep filler pairs for next strip's phase A
                    emit_conv_pair(conv_queue.popleft())
                    break  # at most one pair per iteration

        emit_body()
        if not external_io:
            nc.sync.dma_start(out=outs_d[:], in_=out_d[0:1, 0:8])

    nc.compile()
    return nc


_cache = {}


def _bf16():
    import ml_dtypes
    return ml_dtypes.bfloat16


def get_program(h=256, w=256, r_out=64):
    key = (h, w, r_out)
    if key not in _cache:
        _cache[key] = build_program(h, w, r_out)
    return _cache[key]


def _fp8():
    import ml_dtypes
    return ml_dtypes.float8_e4m3


def make_weight_inputs(prompt, conv_w, b_fc1_w, b_fc1_b, b_fc2_w, b_fc2_b,
                       t_fc1_w, t_fc1_b, t_fc2_w, t_fc2_b):
    f = np.float32
    bf = _bf16()
    wa = np.zeros((128, 14), f)
    wa[:64, :8] = b_fc1_w.T
    wa[64:128, 8:14] = t_fc1_w[:, :64].T
    wb = np.zeros((128, 14), f)
    wb[:64, 8:14] = t_fc1_w[:, 64:].T
    wab = np.zeros((128, 2, 16), f)
    wab[:, 0, :14] = wa
    wab[:, 1, :14] = wb
    wab = wab.reshape(128, 32)
    b1 = np.zeros((14, 1), f)
    b1[:8, 0] = b_fc1_b
    b1[8:14, 0] = t_fc1_b
    mz = np.zeros((14, NTK), f)
    bz = np.zeros((NTK, 1), f)
    for t in range(NT):
        for k in range(NB):
            c = t * NB + k
            mz[:8, c] = b_fc2_w[k, :]
            mz[8:, c] = t_fc2_w[t, :]
            bz[c, 0] = b_fc2_b[k] + t_fc2_b[t]
    return {
        "wab": wab.astype(_fp8()),
        "b1": b1,
        "mz": mz.astype(bf),
        "bz": bz,
        "on": np.ones((NTK, E), bf),
        "pt": np.ascontiguousarray(prompt.reshape(NTK, E)).astype(bf),
        "wt": np.ascontiguousarray(
            conv_w.transpose(2, 3, 1, 0).reshape(9, E, E)).astype(bf),
    }


def make_core_inputs(x_b, flux_b, weights, h, w):
    PIX = h * w
    xfb = np.zeros((128, 2, PIX), np.float32)
    xfb[:DIM, 0] = x_b.reshape(DIM, PIX)
    xfb[DIM:, 0] = flux_b[:64].reshape(64, PIX)
    xfb[:64, 1] = flux_b[64:].reshape(64, PIX)
    m = {"xfb": xfb.astype(_fp8())}
    m.update(weights)
    return m


def kernel(x, flux, prompt, conv_w, b_fc1_w, b_fc1_b, b_fc2_w, b_fc2_b,
           t_fc1_w, t_fc1_b, t_fc2_w, t_fc2_b):
    x = np.asarray(x, np.float32)
    flux = np.asarray(flux, np.float32)
    flux = np.where(np.isnan(flux), np.float32(0), flux)
    h, w = x.shape[2], x.shape[3]

    nc = get_program(h=h, w=w)
    weights = make_weight_inputs(
        np.asarray(prompt, np.float32), np.asarray(conv_w, np.float32),
        np.asarray(b_fc1_w, np.float32), np.asarray(b_fc1_b, np.float32),
        np.asarray(b_fc2_w, np.float32), np.asarray(b_fc2_b, np.float32),
        np.asarray(t_fc1_w, np.float32), np.asarray(t_fc1_b, np.float32),
        np.asarray(t_fc2_w, np.float32), np.asarray(t_fc2_b, np.float32))
    in_maps = [make_core_inputs(x[i], flux[i], weights, h, w)
               for i in range(NCORES)]
    res = run_bass_kernel_spmd(nc, in_maps, list(range(NCORES)))
    out = np.stack([res.results[i]["out"].astype(np.float32).reshape(E, h, w)
                    for i in range(NCORES)], axis=0)
    return out
